# revision 1
# baseline (speedup 1.0000x reference)
"""Binarized-MLP (BNN) kernel for Trainium2, data-parallel over batch on 8 NeuronCores.

Reference computation:
    h      = x @ sign(W1) + b1          x:[8192,4096] W1:[4096,512]
    logits = sign(h) @ sign(W2) + b2    W2:[512,10]
    out    = softmax(logits)            [8192,10]

Device strategy (per core, batch shard of 1024 rows):
  - x is supplied pre-transposed and split hi/lo in bf16 (x = hi + lo to
    ~2^-18 relative accuracy), so the dominant matmul runs as two bf16
    TensorE passes accumulating into the same fp32 PSUM bank — fp32-grade
    accuracy at bf16 speed.
  - Layout: stationary = sign(W1) f-tile [128f x 128j], moving = xT f-tile
    [128f x 512b] -> PSUM [128j x 512b]; all 8 PSUM banks hold the full
    per-core h [512 x 1024] and accumulate across the 32 f-tiles.
  - Inputs are host-packed so four f-tiles arrive per DMA with 8KB
    contiguous per partition line (DMA issue cost here is per-descriptor:
    ~0.6us per 128-partition DMA regardless of bytes).
  - The last four f-tiles run bank-major so each PSUM bank finishes early
    and sign(h)/second-matmul/softmax overlap the remaining big matmuls.
  - sign(h)+b1 is fused into one ScalarE Sign-activation (bias=b1) straight
    out of PSUM into bf16 SBUF tiles, laid out [j, b] as the stationary
    operand of the second matmul. sign() of weights adds a +1e-30 bias so
    sign(0) == +1 like the reference's where(x >= 0) (W1 contains one 0.0).
  - Second matmul: stationary = sign(h) [128j x 128b], moving = sign(W2)
    [128j x 10] accumulated over 4 j-tiles -> PSUM [128b x 10].
  - Softmax on [128b, 10] tiles: add b2 (host-replicated [128,10]),
    reduce_max(negate) -> Exp activation with per-row bias and fused row-sum
    (accum_out), reciprocal, scale into a collect tile, single packed DMA out.
"""

import numpy as np
import ml_dtypes

import concourse.bass as bass
import concourse.tile as tile
from concourse import mybir
from concourse.bass_utils import run_bass_kernel_spmd
from bass_rust import ScopedClock, VectorClock

_CLEAR_SEMS = True

BF16 = mybir.dt.bfloat16
F32 = mybir.dt.float32

B, F, H, C = 8192, 4096, 512, 10
NCORES = 8
BC = B // NCORES          # 1024 batch rows per core
NF = F // 128             # 32 f-tiles (contraction)
NJ = H // 128             # 4 j-tiles (hidden)
NBC = BC // 512           # 2 moving-operand chunks of 512
NBT = BC // 128           # 8 output b-tiles
NQ = NF // 4              # 8 quads of f-tiles (4 per DMA)


class _PatchedTileContext(tile.TileContext):
    """Workaround for the walrus build in this container only accepting one
    sem wait on a CTRL-type (Drain) instruction: spread the exit drain's
    per-proc waits across several drains with one wait each."""

    def _drain_and_barrier(self, tick_clock, wait_clock):
        gc = tick_clock.global_clock
        ticks = list(gc)
        nprocs = len(ticks)
        engines = [
            self.nc.sync,
            self.nc.gpsimd,
            self.nc.vector,
            self.nc.scalar,
            self.nc.tensor,
        ]
        # Cheap wait-carriers: one engine NOP per pending proc tick, spread
        # round-robin so the waits resolve in parallel (a DRAIN costs ~1us on
        # some engines; a NOP ~50ns).
        k = 0
        for i, t in enumerate(ticks):
            if t == 0:
                continue
            partial = [0] * nprocs
            partial[i] = t
            inst = engines[k % len(engines)].nop()
            k += 1
            wait_clock.add_sem_waits(
                inst.ins, ScopedClock({None: VectorClock(partial)})
            )
        self.nc.sync.drain()

        self.nc.all_engine_barrier()
        assert self.sems is not None
        popped = self.nc._tile_sem_poison_stack.pop()
        assert popped is self._sem_poison
        if _CLEAR_SEMS:
            # gpsimd-only cleanup (range-clear is a single op there); no
            # closing barrier — each engine halts after its own stream, and
            # NEFF completion waits for all engines anyway.
            self.nc.clear_and_free_semaphores(list(self.sems.allocated().values()))


def _split_waits_json(raw: bytes) -> bytes:
    """The walrus build in this container accepts at most ONE sem wait per
    instruction (bass's own wait_op asserts the same). Tile attaches several.
    Rewrite the serialized BIR: excess waits become standalone EventSemaphore
    wait instructions on the same engine immediately before the instruction —
    semantically identical, since the engine blocks there first."""
    import json as _json

    m = _json.loads(raw)
    ctr = 0
    for fn in m.get("functions", []):
        for bb in fn.get("blocks", []):
            insts = bb.get("instructions", [])
            new_insts = []
            for inst in insts:
                si = inst.get("sync_info")
                waits = si.get("on_wait") or [] if si else []
                if len(waits) > 1:
                    for w in waits[:-1]:
                        new_insts.append(
                            {
                                "debug": inst.get("debug", 0),
                                "engine": inst["engine"],
                                "ins": [],
                                "outs": [],
                                "name": f"WSPLIT-{ctr}",
                                "opcode": "EventSemaphore",
                                "sync_info": {"on_update": [], "on_wait": [w]},
                            }
                        )
                        ctr += 1
                    si["on_wait"] = [waits[-1]]
                new_insts.append(inst)
            bb["instructions"] = new_insts
    return _json.dumps(m).encode()


def _install_wait_splitter(nc: bass.Bass) -> None:
    orig = nc.to_json_bytes

    def patched():
        return _split_waits_json(orig())

    nc.to_json_bytes = patched


def build_kernel() -> bass.Bass:
    nc = bass.Bass()
    # Quad-packed streams: row q*128+p holds 4 f-subtiles contiguously.
    # xtq sub-layout per row: [i=0..3][hi 1024 | lo 1024]  (8KB / partition line)
    xtq = nc.dram_tensor("xtq", [NQ * 128, 4 * 2 * BC], BF16, kind="ExternalInput")
    # w1q sub-layout per row: [i=0..3][512 h-cols]          (4KB / partition line)
    w1q = nc.dram_tensor("w1q", [NQ * 128, 4 * H], BF16, kind="ExternalInput")
    b1p = nc.dram_tensor("b1p", [128, NJ], F32, kind="ExternalInput")
    w2p = nc.dram_tensor("w2p", [128, NJ * C], F32, kind="ExternalInput")
    b2r = nc.dram_tensor("b2r", [128, C], F32, kind="ExternalInput")
    # packed per-core output [p, bt*10+c]; host reorders to [1024, 10]
    out = nc.dram_tensor("out", [128, NBT * C], F32, kind="ExternalOutput")

    with _PatchedTileContext(nc) as tc:
        with (
            tc.tile_pool(name="consts", bufs=1) as consts,
            tc.tile_pool(name="w1raw", bufs=2) as w1raw_pool,
            tc.tile_pool(name="w1s", bufs=2) as w1s_pool,
            tc.tile_pool(name="xin", bufs=12) as xin_pool,
            tc.tile_pool(name="signh", bufs=NJ * NBC) as signh_pool,
            tc.tile_pool(name="psum", bufs=8, space="PSUM") as psum_pool,
            tc.tile_pool(name="smx", bufs=4) as smx_pool,
        ):
            tiny = consts.tile([128, 1], F32, name="tiny", tag="tiny")
            nc.vector.memset(tiny[:], 1e-30)

            psumB = [
                [psum_pool.tile([128, 512], F32, name="psB", tag="psB") for _ in range(NBC)]
                for _ in range(NJ)
            ]

            # HAM warmup: PE sits idle ~4us while the first tiles land; a
            # dozen dummy matmuls (into bank 0, overwritten by the first real
            # start=True matmul) bring the clock gate to 2.4GHz beforehand.
            warm = consts.tile([128, 640], BF16, name="warm", tag="warm")
            nc.vector.memset(warm[:], 0.0)
            for _ in range(14):
                nc.tensor.matmul(
                    psumB[0][0][:], warm[:, :128], warm[:, 128:640],
                    start=True, stop=True,
                )

            def quad_in(q):
                w1s = w1s_pool.tile([128, 2048], BF16, name="w1s", tag="w1s")
                rowq = w1q[q * 128:(q + 1) * 128, :]
                if q == 0:
                    # the first moving-operand halves go out first: the sign
                    # chain runs on ACT while they transfer, so the first
                    # matmul is gated by whichever lands last
                    xrow0 = xtq[0:128, :]
                    xh0 = xin_pool.tile([128, 1024], BF16, name="xin0h", tag="xin0h")
                    nc.sync.dma_start(xh0[:], xrow0[:, 0:1024])
                    xl0 = xin_pool.tile([128, 1024], BF16, name="xin0l", tag="xin0l")
                    nc.sync.dma_start(xl0[:], xrow0[:, 1024:2048])
                    # startup: smaller first transfers so the first sign/MM
                    # start ~4us earlier (one 1MB DMA takes ~3us to land)
                    for i in range(4):
                        raw = w1raw_pool.tile([128, 512], BF16, name="w1raw", tag="w1raw")
                        nc.sync.dma_start(raw[:], rowq[:, i * 512:(i + 1) * 512])
                        nc.scalar.sign(
                            w1s[:, i * 512:(i + 1) * 512], raw[:], bias=tiny[:]
                        )
                else:
                    raw = w1raw_pool.tile([128, 2048], BF16, name="w1raw", tag="w1raw")
                    nc.sync.dma_start(raw[:], rowq)
                    for i in range(4):
                        nc.scalar.sign(
                            w1s[:, i * 512:(i + 1) * 512],
                            raw[:, i * 512:(i + 1) * 512],
                            bias=tiny[:],
                        )
                xrowq = xtq[q * 128:(q + 1) * 128, :]
                xfs = []
                for i in range(4):
                    if q == 0 and i == 0:
                        xfs.append((xh0, xl0))
                    else:
                        xf = xin_pool.tile([128, 2048], BF16, name="xin", tag="xin")
                        nc.sync.dma_start(
                            xf[:], xrowq[:, i * 2048:(i + 1) * 2048]
                        )
                        xfs.append(xf)
                return w1s, xfs

            def quad_mms(w1s, xfs, i, j, bc, start, stop):
                lhs = w1s[:, i * 512 + j * 128:i * 512 + (j + 1) * 128]
                xf = xfs[i]
                if isinstance(xf, tuple):
                    hi = xf[0][:, bc * 512:(bc + 1) * 512]
                    lo = xf[1][:, bc * 512:(bc + 1) * 512]
                else:
                    hi = xf[:, bc * 512:(bc + 1) * 512]
                    lo = xf[:, 1024 + bc * 512:1024 + (bc + 1) * 512]
                nc.tensor.matmul(psumB[j][bc][:], lhs, hi, start=start, stop=False)
                nc.tensor.matmul(psumB[j][bc][:], lhs, lo, start=False, stop=stop)

            # ---- phase 1: quads 0..NQ-2, f-major over all 8 banks ----
            b1_t = w2raw = w2s = b2_t = None
            for q in range(NQ - 1):
                if q == 0:
                    with tc.high_priority():
                        w1s, xf = quad_in(q)
                else:
                    w1s, xf = quad_in(q)
                if q == 0:
                    # constants: packed, one DMA each, after the first quad's
                    # stream DMAs so they stay off the startup critical path
                    b1_t = consts.tile([128, NJ], F32, name="b1t", tag="b1t")
                    nc.sync.dma_start(b1_t[:], b1p[:, :])
                    w2raw = consts.tile([128, NJ * C], F32, name="w2raw", tag="w2raw")
                    nc.sync.dma_start(w2raw[:], w2p[:, :])
                    b2_t = consts.tile([128, C], F32, name="b2", tag="b2")
                    nc.sync.dma_start(b2_t[:], b2r[:, :])
                for i in range(4):
                    for j in range(NJ):
                        for bc in range(NBC):
                            quad_mms(w1s, xf, i, j, bc,
                                     start=(q == 0 and i == 0), stop=False)

            # ---- phase 2: last quad bank-major; sign/mm2/softmax overlap ----
            w1s_l, xf_l = quad_in(NQ - 1)
            w2s = consts.tile([128, NJ * C], BF16, name="w2s", tag="w2s")
            nc.scalar.sign(w2s[:], w2raw[:], bias=tiny[:])
            signh = [[None] * NBC for _ in range(NJ)]
            collect = smx_pool.tile([128, NBT * C], F32, name="collect", tag="collect")
            for bc in range(NBC):
                for j in range(NJ):
                    for i in range(4):
                        quad_mms(w1s_l, xf_l, i, j, bc,
                                 start=False, stop=(i == 3))
                    s = signh_pool.tile([128, 512], BF16, name="signh", tag="signh")
                    nc.scalar.sign(s[:], psumB[j][bc][:], bias=b1_t[:, j:j + 1])
                    signh[j][bc] = s
                for bt in range(bc * 4, bc * 4 + 4):
                    col = (bt % 4) * 128
                    ps2 = psum_pool.tile([128, C], F32, name="psD", tag="psB")
                    for j in range(NJ):
                        nc.tensor.matmul(
                            ps2[:],
                            signh[j][bc][:, col:col + 128],
                            w2s[:, j * C:(j + 1) * C],
                            start=(j == 0),
                            stop=(j == NJ - 1),
                        )
                    logits = smx_pool.tile([128, C], F32, name="logits", tag="logits")
                    nc.vector.tensor_add(logits[:], ps2[:], b2_t[:])
                    negmax = smx_pool.tile([128, 1], F32, name="negmax", tag="negmax")
                    nc.vector.reduce_max(
                        negmax[:], logits[:], axis=mybir.AxisListType.X, negate=True
                    )
                    e = smx_pool.tile([128, C], F32, name="e", tag="e")
                    ssum = smx_pool.tile([128, 1], F32, name="ssum", tag="ssum")
                    nc.scalar.activation(
                        e[:],
                        logits[:],
                        mybir.ActivationFunctionType.Exp,
                        bias=negmax[:],
                        accum_out=ssum[:],
                    )
                    lns = smx_pool.tile([128, 1], F32, name="lns", tag="lns")
                    nc.scalar.activation(
                        lns[:], ssum[:], mybir.ActivationFunctionType.Ln
                    )
                    negms = smx_pool.tile([128, 1], F32, name="negms", tag="negms")
                    nc.vector.tensor_sub(negms[:], negmax[:], lns[:])
                    nc.scalar.activation(
                        collect[:, bt * C:(bt + 1) * C],
                        logits[:],
                        mybir.ActivationFunctionType.Exp,
                        bias=negms[:],
                    )

            # per-chunk output DMAs; host unpacks [p, bt*10+c] -> [bt*128+p, c].
            # bc0's half leaves while bc1 computes, so only a 20KB transfer
            # trails the final softmax
            half = 4 * C
            nc.sync.dma_start(out[:, 0:half], collect[:, 0:half])
            nc.sync.dma_start(out[:, half:2 * half], collect[:, half:2 * half])

    _install_wait_splitter(nc)
    return nc


_cached_nc = None


def _get_nc() -> bass.Bass:
    global _cached_nc
    if _cached_nc is None:
        _cached_nc = build_kernel()
    return _cached_nc


def kernel(inputs, W1, b1, W2, b2):
    x = np.ascontiguousarray(np.asarray(inputs, dtype=np.float32))
    W1 = np.asarray(W1, dtype=np.float32)
    b1 = np.asarray(b1, dtype=np.float32)
    W2 = np.asarray(W2, dtype=np.float32)
    b2 = np.asarray(b2, dtype=np.float32)

    w1_bf = W1.astype(ml_dtypes.bfloat16)
    # [4096, 512] -> quad-packed [NQ*128, 4*512]
    w1_pack = np.ascontiguousarray(
        w1_bf.reshape(NQ, 4, 128, H).transpose(0, 2, 1, 3).reshape(NQ * 128, 4 * H)
    )
    b1_pack = np.ascontiguousarray(b1.reshape(NJ, 128).T)
    w2_pack = np.ascontiguousarray(
        W2.reshape(NJ, 128, C).transpose(1, 0, 2).reshape(128, NJ * C)
    )
    b2_rep = np.ascontiguousarray(np.broadcast_to(b2.reshape(1, C), (128, C)))

    in_maps = []
    for c in range(NCORES):
        xc_t = x[c * BC:(c + 1) * BC, :].T  # [F, BC]
        hi = xc_t.astype(ml_dtypes.bfloat16)
        lo = (xc_t - hi.astype(np.float32)).astype(ml_dtypes.bfloat16)
        pack = np.empty((NQ, 128, 4, 2, BC), dtype=ml_dtypes.bfloat16)
        pack[:, :, :, 0] = hi.reshape(NQ, 4, 128, BC).transpose(0, 2, 1, 3)
        pack[:, :, :, 1] = lo.reshape(NQ, 4, 128, BC).transpose(0, 2, 1, 3)
        in_maps.append(
            {
                "xtq": pack.reshape(NQ * 128, 4 * 2 * BC),
                "w1q": w1_pack,
                "w2p": w2_pack,
                "b1p": b1_pack,
                "b2r": b2_rep,
            }
        )

    nc = _get_nc()
    res = run_bass_kernel_spmd(nc, in_maps, core_ids=list(range(NCORES)))
    global last_results
    last_results = res
    parts = []
    for c in range(NCORES):
        oc = res.results[c]["out"]  # [128, NBT*C]
        parts.append(
            oc.reshape(128, NBT, C).transpose(1, 0, 2).reshape(BC, C)
        )
    return np.concatenate(parts, axis=0).astype(np.float32)


last_results = None



# revision 5
# speedup vs baseline: 1.1810x; 1.1810x over previous
"""Binarized-MLP (BNN) kernel for Trainium2, data-parallel over batch on 8 NeuronCores.

Reference computation:
    h      = x @ sign(W1) + b1          x:[8192,4096] W1:[4096,512]
    logits = sign(h) @ sign(W2) + b2    W2:[512,10]
    out    = softmax(logits)            [8192,10]

Per-core strategy (batch shard of 1024 rows):
  - The dominant matmul runs as a fp16 "hi" pass plus an fp8-e4m3 "lo"
    residual pass in DoubleRow perf mode (2 fp8 rows per PE cell, 256-row
    contraction per matmul). lo = (x - fp16(x)) * 256 is host-quantized to
    e4m3; the 1/64 un-scale is folded into the lo-pass stationary weights
    (sign(W1) * 2^-8, exact as e4m3 subnormal; verified exact on HW). Both passes accumulate into the same
    fp32 PSUM banks. End-to-end rel err ~9e-3 (gate 2e-2), measured in
    numpy with bit-identical host quantization.
  - Weights are sign()ed on the host (free) — no device-side weight prep.
  - hi pass: stationary = sign(W1) f-tile [128f x 128j], moving = xT hi
    f-tile [128f x 512b] -> 8 PSUM banks hold h [512j x 1024b]; 256 MMs,
    f-major, fully dense on TensorE.
  - lo pass: stationary = 3D AP [128, 2, 128] fp8 pair-tile, moving =
    [128, 2, 512] fp8 -> 128 DoubleRow MMs, bank-major so each PSUM bank
    finishes early and sign/mm2/softmax overlap the remaining MMs.
  - sign(h)+b1 fused into one ScalarE Sign-activation (bias=b1) from PSUM
    into bf16 SBUF tiles [j, b], the stationary operand of the 2nd matmul.
  - 2nd matmul accumulates incrementally after each sign (j-wise) into a
    single PSUM bank holding all 4 b-tiles' logits [128, 4, 10].
  - softmax: add b2 + segmented reduce_max over [128,4,10] once, then per
    b-tile: Exp(bias=-max, accum_out=sum) -> DVE reciprocal -> scalar-mul,
    with a per-b-tile output DMA so only ~5KB trails the last chain.
"""

import numpy as np
import ml_dtypes

import concourse.bass as bass
import concourse.tile as tile
from concourse import mybir
from concourse.bass_utils import run_bass_kernel_spmd
from bass_rust import ScopedClock, VectorClock

_CLEAR_SEMS = True

BF16 = mybir.dt.bfloat16
FP16 = mybir.dt.float16
F32 = mybir.dt.float32
FP8 = mybir.dt.float8e4

B, F, H, C = 8192, 4096, 512, 10
NCORES = 8
BC = B // NCORES          # 1024 batch rows per core
NF = F // 128             # 32 f-tiles (contraction)
NJ = H // 128             # 4 j-tiles (hidden)
NBC = BC // 512           # 2 moving-operand chunks of 512
NBT = BC // 128           # 8 output b-tiles
NQ = NF // 4              # 8 hi-pass quads (4 f-tiles per DMA)
NP = NF // 2              # 16 lo-pass pair-tiles (DoubleRow: 2 f-tiles/MM)

LO_SCALE = 256.0          # lo residual pre-scale (dodges e4m3 subnormals)
W_LO = 1.0 / LO_SCALE     # folded into lo-pass weights; 2^-8 exact in e4m3 (subnormal)


class _PatchedTileContext(tile.TileContext):
    """Workaround for the walrus build in this container only accepting one
    sem wait on a CTRL-type (Drain) instruction: spread the exit drain's
    per-proc waits across several drains with one wait each."""

    def _drain_and_barrier(self, tick_clock, wait_clock):
        gc = tick_clock.global_clock
        ticks = list(gc)
        nprocs = len(ticks)
        engines = [
            self.nc.sync,
            self.nc.gpsimd,
            self.nc.vector,
            self.nc.scalar,
            self.nc.tensor,
        ]
        # Cheap wait-carriers: one engine NOP per pending proc tick, spread
        # round-robin so the waits resolve in parallel (a DRAIN costs ~1us on
        # some engines; a NOP ~50ns).
        k = 0
        for i, t in enumerate(ticks):
            if t == 0:
                continue
            partial = [0] * nprocs
            partial[i] = t
            inst = engines[k % len(engines)].nop()
            k += 1
            wait_clock.add_sem_waits(
                inst.ins, ScopedClock({None: VectorClock(partial)})
            )
        self.nc.sync.drain()

        self.nc.all_engine_barrier()
        assert self.sems is not None
        popped = self.nc._tile_sem_poison_stack.pop()
        assert popped is self._sem_poison
        if _CLEAR_SEMS:
            # gpsimd-only cleanup (range-clear is a single op there); no
            # closing barrier — each engine halts after its own stream, and
            # NEFF completion waits for all engines anyway.
            self.nc.clear_and_free_semaphores(list(self.sems.allocated().values()))


def _split_waits_json(raw: bytes) -> bytes:
    """The walrus build in this container accepts at most ONE sem wait per
    instruction (bass's own wait_op asserts the same). Tile attaches several.
    Rewrite the serialized BIR: excess waits become standalone EventSemaphore
    wait instructions on the same engine immediately before the instruction —
    semantically identical, since the engine blocks there first."""
    import json as _json

    m = _json.loads(raw)
    ctr = 0
    for fn in m.get("functions", []):
        for bb in fn.get("blocks", []):
            insts = bb.get("instructions", [])
            new_insts = []
            for inst in insts:
                si = inst.get("sync_info")
                waits = si.get("on_wait") or [] if si else []
                if len(waits) > 1:
                    for w in waits[:-1]:
                        new_insts.append(
                            {
                                "debug": inst.get("debug", 0),
                                "engine": inst["engine"],
                                "ins": [],
                                "outs": [],
                                "name": f"WSPLIT-{ctr}",
                                "opcode": "EventSemaphore",
                                "sync_info": {"on_update": [], "on_wait": [w]},
                            }
                        )
                        ctr += 1
                    si["on_wait"] = [waits[-1]]
                new_insts.append(inst)
            bb["instructions"] = new_insts
    return _json.dumps(m).encode()


def _install_wait_splitter(nc: bass.Bass) -> None:
    orig = nc.to_json_bytes

    def patched():
        return _split_waits_json(orig())

    nc.to_json_bytes = patched


def build_kernel() -> bass.Bass:
    nc = bass.Bass()
    # hi quad-packed: row q*128+p holds [i=0..3][1024 b cols] bf16 (8KB/row)
    xhq = nc.dram_tensor("xhq", [NQ * 128, 4 * BC], FP16, kind="ExternalInput")
    # lo pair-packed: row t*128+p holds [i=0..1][1024 b cols] e4m3 (2KB/row)
    xlp = nc.dram_tensor("xlp", [NP * 128, 2 * BC], FP8, kind="ExternalInput")
    # sign(W1) bf16 quad-packed: row q*128+p holds [i=0..3][512 j] (4KB/row)
    w1h = nc.dram_tensor("w1h", [NQ * 128, 4 * H], FP16, kind="ExternalInput")
    # sign(W1)*2^-6 e4m3, all pair-tiles per partition: [t=0..15][i=0..1][512 j]
    w1l = nc.dram_tensor("w1l", [128, NP * 2 * H], FP8, kind="ExternalInput")
    b1p = nc.dram_tensor("b1p", [128, NJ], F32, kind="ExternalInput")
    # sign(W2) bf16: w2s[p, j*C+c] = sign(W2)[j*128+p, c]
    w2sd = nc.dram_tensor("w2sd", [128, NJ * C], BF16, kind="ExternalInput")
    # b2 replicated [128, 4, C] for the fused [128,4,10] bias add
    b2r4 = nc.dram_tensor("b2r4", [128, 4 * C], F32, kind="ExternalInput")
    # packed per-core output [p, bt*10+c]; host reorders to [1024, 10]
    out = nc.dram_tensor("out", [128, NBT * C], F32, kind="ExternalOutput")

    with _PatchedTileContext(nc) as tc:
        with (
            tc.tile_pool(name="consts", bufs=1) as consts,
            tc.tile_pool(name="w1hp", bufs=NQ) as w1h_pool,
            tc.tile_pool(name="xh", bufs=NQ) as xh_pool,
            tc.tile_pool(name="xl", bufs=NP) as xl_pool,
            tc.tile_pool(name="signh", bufs=NJ * NBC) as signh_pool,
            tc.tile_pool(name="psum", bufs=8, space="PSUM") as psum_pool,
            tc.tile_pool(name="smx", bufs=10) as smx_pool,
        ):
            # allocation order bc-major: pool ring slots 0-3 = bc0 banks,
            # 4-7 = bc1, so each bc's four psD re-allocations alias banks
            # already freed by that bc's own sign() reads.
            _ps = [
                psum_pool.tile([128, 512], F32, name="psB", tag="psB")
                for _ in range(NJ * NBC)
            ]
            psumB = [[_ps[bc * NJ + j] for bc in range(NBC)] for j in range(NJ)]

            # HAM warmup: PE sits idle ~4us while the first tiles land; a
            # dozen dummy matmuls (into bank 0, overwritten by the first real
            # start=True matmul) bring the clock gate to 2.4GHz beforehand.
            warm = consts.tile([128, 640], FP16, name="warm", tag="warm")
            nc.vector.memset(warm[:], 0.0)
            for _ in range(14):
                nc.tensor.matmul(
                    psumB[0][0][:], warm[:, :128], warm[:, 128:640],
                    start=True, stop=True,
                )

            def quad_in(q):
                w1t = w1h_pool.tile([128, 4 * 512], FP16, name="w1t", tag="w1t")
                xht = xh_pool.tile([128, 4 * BC], FP16, name="xht", tag="xht")
                roww = w1h[q * 128:(q + 1) * 128, :]
                rowx = xhq[q * 128:(q + 1) * 128, :]
                if q == 0:
                    # startup: small transfers first so the first MM starts
                    # as early as possible (i=0 weights, then i=0/bc=0 x)
                    nc.sync.dma_start(w1t[:, 0:512], roww[:, 0:512])
                    nc.sync.dma_start(xht[:, 0:512], rowx[:, 0:512])
                    nc.sync.dma_start(xht[:, 512:1024], rowx[:, 512:1024])
                    nc.sync.dma_start(w1t[:, 512:2048], roww[:, 512:2048])
                    nc.sync.dma_start(xht[:, 1024:4096], rowx[:, 1024:4096])
                else:
                    nc.sync.dma_start(w1t[:], roww)
                    nc.sync.dma_start(xht[:], rowx)
                return w1t, xht

            # ---- hi pass: bf16, f-major over all 8 banks ----
            b1_t = w2s = b2q = w1lt = None
            xlt = [None] * NP
            for q in range(NQ):
                if q == 0:
                    with tc.high_priority():
                        w1t, xht = quad_in(q)
                else:
                    w1t, xht = quad_in(q)
                if q == 0:
                    # constants: packed, one DMA each, after the first quad's
                    # stream DMAs so they stay off the startup critical path
                    b1_t = consts.tile([128, NJ], F32, name="b1t", tag="b1t")
                    nc.sync.dma_start(b1_t[:], b1p[:, :])
                    w2s = consts.tile([128, NJ * C], BF16, name="w2s", tag="w2s")
                    nc.sync.dma_start(w2s[:], w2sd[:, :])
                    b2q = consts.tile([128, 4, C], F32, name="b2q", tag="b2q")
                    nc.sync.dma_start(b2q[:], b2r4[:, :])
                if q == 2:
                    # lo-pass weights: 2MB in two DMAs, needed at ~60us
                    w1lt = consts.tile([128, NP, 2, H], FP8, name="w1lt", tag="w1lt")
                    half = NP * H  # bytes per half row
                    nc.sync.dma_start(w1lt[:, 0:NP // 2], w1l[:, 0:half])
                    nc.sync.dma_start(w1lt[:, NP // 2:NP], w1l[:, half:2 * half])
                for i in range(4):
                    for j in range(NJ):
                        for bc in range(NBC):
                            nc.tensor.matmul(
                                psumB[j][bc][:],
                                w1t[:, i * 512 + j * 128:i * 512 + (j + 1) * 128],
                                xht[:, i * BC + bc * 512:i * BC + (bc + 1) * 512],
                                start=(q == 0 and i == 0),
                                stop=False,
                            )
                # lo-pass x: 2 pair-tiles per quad, streamed behind the quad
                for t in (2 * q, 2 * q + 1):
                    xl = xl_pool.tile([128, 2, BC], FP8, name="xl", tag="xl")
                    nc.sync.dma_start(xl[:], xlp[t * 128:(t + 1) * 128, :])
                    xlt[t] = xl

            # ---- lo pass: fp8 DoubleRow, bank-major; sign/mm2/softmax
            #      overlap the remaining DR MMs ----
            collect = smx_pool.tile([128, NBT * C], F32, name="collect", tag="collect")
            for bc in range(NBC):
                signh = [None] * NJ
                for j in range(NJ):
                    for t in range(NP):
                        nc.tensor.matmul(
                            psumB[j][bc][:],
                            w1lt[:, t, :, j * 128:(j + 1) * 128],
                            xlt[t][:, :, bc * 512:(bc + 1) * 512],
                            start=False,
                            stop=(t == NP - 1),
                            perf_mode=mybir.MatmulPerfMode.DoubleRow,
                        )
                    s = signh_pool.tile([128, 512], BF16, name="signh", tag="signh")
                    nc.scalar.sign(s[:], psumB[j][bc][:], bias=b1_t[:, j:j + 1])
                    signh[j] = s
                for bt in range(4):
                    gbt = bc * 4 + bt
                    # one bank per b-tile (PSUM start=True zeroing is coarser
                    # than 40B, so logit groups can't share a bank); aliases a
                    # bank this bc's signs already freed
                    ps2 = psum_pool.tile([128, C], F32, name="psD", tag="psB")
                    for j in range(NJ):
                        nc.tensor.matmul(
                            ps2[:],
                            signh[j][:, bt * 128:(bt + 1) * 128],
                            w2s[:, j * C:(j + 1) * C],
                            start=(j == 0),
                            stop=(j == NJ - 1),
                        )
                    logits = smx_pool.tile([128, C], F32, name="logits", tag="logits")
                    nc.vector.tensor_add(logits[:], ps2[:], b2q[:, 0])
                    negmax = smx_pool.tile([128, 1], F32, name="negmax", tag="negmax")
                    nc.vector.reduce_max(
                        negmax[:], logits[:], axis=mybir.AxisListType.X, negate=True
                    )
                    e = smx_pool.tile([128, C], F32, name="e", tag="e")
                    ssum = smx_pool.tile([128, 1], F32, name="ssum", tag="ssum")
                    nc.scalar.activation(
                        e[:],
                        logits[:],
                        mybir.ActivationFunctionType.Exp,
                        bias=negmax[:],
                        accum_out=ssum[:],
                    )
                    rinv = smx_pool.tile([128, 1], F32, name="rinv", tag="rinv")
                    nc.vector.reciprocal(rinv[:], ssum[:])
                    nc.vector.tensor_scalar_mul(
                        collect[:, gbt * C:(gbt + 1) * C],
                        e[:],
                        rinv[:],
                    )
                    nc.sync.dma_start(
                        out[:, gbt * C:(gbt + 1) * C],
                        collect[:, gbt * C:(gbt + 1) * C],
                    )

    _install_wait_splitter(nc)
    return nc


_cached_nc = None


def _get_nc() -> bass.Bass:
    global _cached_nc
    if _cached_nc is None:
        _cached_nc = build_kernel()
    return _cached_nc


def kernel(inputs, W1, b1, W2, b2):
    x = np.ascontiguousarray(np.asarray(inputs, dtype=np.float32))
    W1 = np.asarray(W1, dtype=np.float32)
    b1 = np.asarray(b1, dtype=np.float32)
    W2 = np.asarray(W2, dtype=np.float32)
    b2 = np.asarray(b2, dtype=np.float32)

    w1s = np.where(W1 >= 0, np.float32(1.0), np.float32(-1.0))
    # [4096, 512] -> quad-packed [NQ*128, 4*512] bf16
    w1h_pack = np.ascontiguousarray(
        w1s.astype(np.float16)
        .reshape(NQ, 4, 128, H).transpose(0, 2, 1, 3).reshape(NQ * 128, 4 * H)
    )
    # lo-pass weights: sign(W1)*2^-6, per-partition [t][i][512 j]
    w1l_pack = np.ascontiguousarray(
        (w1s * W_LO).astype(ml_dtypes.float8_e4m3)
        .reshape(NP, 2, 128, H).transpose(2, 0, 1, 3).reshape(128, NP * 2 * H)
    )
    b1_pack = np.ascontiguousarray(b1.reshape(NJ, 128).T)
    w2s_pack = np.ascontiguousarray(
        np.where(W2 >= 0, np.float32(1.0), np.float32(-1.0))
        .astype(ml_dtypes.bfloat16)
        .reshape(NJ, 128, C).transpose(1, 0, 2).reshape(128, NJ * C)
    )
    b2_rep4 = np.ascontiguousarray(
        np.broadcast_to(b2.reshape(1, 1, C), (128, 4, C)).reshape(128, 4 * C)
    )

    in_maps = []
    for c in range(NCORES):
        xc_t = x[c * BC:(c + 1) * BC, :].T  # [F, BC] fp32
        hi = xc_t.astype(np.float16)
        lo = (xc_t - hi.astype(np.float32)) * LO_SCALE
        lo8 = lo.astype(ml_dtypes.float8_e4m3)
        xh_pack = np.ascontiguousarray(
            hi.reshape(NQ, 4, 128, BC).transpose(0, 2, 1, 3).reshape(NQ * 128, 4 * BC)
        )
        xl_pack = np.ascontiguousarray(
            lo8.reshape(NP, 2, 128, BC).transpose(0, 2, 1, 3).reshape(NP * 128, 2 * BC)
        )
        in_maps.append(
            {
                "xhq": xh_pack,
                "xlp": xl_pack,
                "w1h": w1h_pack,
                "w1l": w1l_pack,
                "b1p": b1_pack,
                "w2sd": w2s_pack,
                "b2r4": b2_rep4,
            }
        )

    nc = _get_nc()
    res = run_bass_kernel_spmd(nc, in_maps, core_ids=list(range(NCORES)))
    global last_results
    last_results = res
    parts = []
    for c in range(NCORES):
        oc = res.results[c]["out"]  # [128, NBT*C]
        parts.append(
            oc.reshape(128, NBT, C).transpose(1, 0, 2).reshape(BC, C)
        )
    return np.concatenate(parts, axis=0).astype(np.float32)


last_results = None


# revision 7
# speedup vs baseline: 1.2617x; 1.0684x over previous
"""Binarized-MLP (BNN) kernel for Trainium2, data-parallel over batch on 8 NeuronCores.

Reference computation:
    h      = x @ sign(W1) + b1          x:[8192,4096] W1:[4096,512]
    logits = sign(h) @ sign(W2) + b2    W2:[512,10]
    out    = softmax(logits)            [8192,10]

Per-core strategy (batch shard of 1024 rows):
  - The dominant matmul runs as a fp16 "hi" pass plus an fp8-e4m3 "lo"
    residual pass in DoubleRow perf mode (2 fp8 rows per PE cell, 256-row
    contraction per matmul). lo = (x - fp16(x)) * 256 is host-quantized to
    e4m3; the 1/64 un-scale is folded into the lo-pass stationary weights
    (sign(W1) * 2^-8, exact as e4m3 subnormal; verified exact on HW). Both passes accumulate into the same
    fp32 PSUM banks. End-to-end rel err ~9e-3 (gate 2e-2), measured in
    numpy with bit-identical host quantization.
  - Weights are sign()ed on the host (free) — no device-side weight prep.
  - hi pass: stationary = sign(W1) f-tile [128f x 128j], moving = xT hi
    f-tile [128f x 512b] -> 8 PSUM banks hold h [512j x 1024b]; 256 MMs,
    f-major, fully dense on TensorE.
  - lo pass: stationary = 3D AP [128, 2, 128] fp8 pair-tile, moving =
    [128, 2, 512] fp8 -> 128 DoubleRow MMs, bank-major so each PSUM bank
    finishes early and sign/mm2/softmax overlap the remaining MMs.
  - sign(h)+b1 fused into one ScalarE Sign-activation (bias=b1) from PSUM
    into bf16 SBUF tiles [j, b], the stationary operand of the 2nd matmul.
  - 2nd matmul accumulates incrementally after each sign (j-wise) into a
    single PSUM bank holding all 4 b-tiles' logits [128, 4, 10].
  - softmax: add b2 + segmented reduce_max over [128,4,10] once, then per
    b-tile: Exp(bias=-max, accum_out=sum) -> DVE reciprocal -> scalar-mul,
    with a per-b-tile output DMA so only ~5KB trails the last chain.
"""

import numpy as np
import ml_dtypes

import concourse.bass as bass
import concourse.tile as tile
from concourse import mybir
from concourse.bass_utils import run_bass_kernel_spmd
from bass_rust import ScopedClock, VectorClock

_CLEAR_SEMS = True

BF16 = mybir.dt.bfloat16
FP16 = mybir.dt.float16
F32 = mybir.dt.float32
FP8 = mybir.dt.float8e4

B, F, H, C = 8192, 4096, 512, 10
NCORES = 8
BC = B // NCORES          # 1024 batch rows per core
NF = F // 128             # 32 f-tiles (contraction)
NJ = H // 128             # 4 j-tiles (hidden)
NBC = BC // 512           # 2 moving-operand chunks of 512
NBT = BC // 128           # 8 output b-tiles
NQ = NF // 4              # 8 hi-pass quads (4 f-tiles per DMA)
NP = NF // 2              # 16 lo-pass pair-tiles (DoubleRow: 2 f-tiles/MM)

LO_SCALE = 256.0          # lo residual pre-scale (dodges e4m3 subnormals)
W_LO = 1.0 / LO_SCALE     # folded into lo-pass weights; 2^-8 exact in e4m3 (subnormal)


class _PatchedTileContext(tile.TileContext):
    """Workaround for the walrus build in this container only accepting one
    sem wait on a CTRL-type (Drain) instruction: spread the exit drain's
    per-proc waits across several drains with one wait each."""

    def _drain_and_barrier(self, tick_clock, wait_clock):
        gc = tick_clock.global_clock
        ticks = list(gc)
        nprocs = len(ticks)
        engines = [
            self.nc.sync,
            self.nc.gpsimd,
            self.nc.vector,
            self.nc.scalar,
            self.nc.tensor,
        ]
        # Cheap wait-carriers: one engine NOP per pending proc tick, spread
        # round-robin so the waits resolve in parallel (a DRAIN costs ~1us on
        # some engines; a NOP ~50ns).
        k = 0
        for i, t in enumerate(ticks):
            if t == 0:
                continue
            partial = [0] * nprocs
            partial[i] = t
            inst = engines[k % len(engines)].nop()
            k += 1
            wait_clock.add_sem_waits(
                inst.ins, ScopedClock({None: VectorClock(partial)})
            )
        self.nc.sync.drain()

        self.nc.all_engine_barrier()
        assert self.sems is not None
        popped = self.nc._tile_sem_poison_stack.pop()
        assert popped is self._sem_poison
        if _CLEAR_SEMS:
            # gpsimd-only cleanup (range-clear is a single op there); no
            # closing barrier — each engine halts after its own stream, and
            # NEFF completion waits for all engines anyway.
            self.nc.clear_and_free_semaphores(list(self.sems.allocated().values()))


def _split_waits_json(raw: bytes) -> bytes:
    """The walrus build in this container accepts at most ONE sem wait per
    instruction (bass's own wait_op asserts the same). Tile attaches several.
    Rewrite the serialized BIR: excess waits become standalone EventSemaphore
    wait instructions on the same engine immediately before the instruction —
    semantically identical, since the engine blocks there first."""
    import json as _json

    m = _json.loads(raw)
    ctr = 0
    for fn in m.get("functions", []):
        for bb in fn.get("blocks", []):
            insts = bb.get("instructions", [])
            new_insts = []
            for inst in insts:
                si = inst.get("sync_info")
                waits = si.get("on_wait") or [] if si else []
                if len(waits) > 1:
                    for w in waits[:-1]:
                        new_insts.append(
                            {
                                "debug": inst.get("debug", 0),
                                "engine": inst["engine"],
                                "ins": [],
                                "outs": [],
                                "name": f"WSPLIT-{ctr}",
                                "opcode": "EventSemaphore",
                                "sync_info": {"on_update": [], "on_wait": [w]},
                            }
                        )
                        ctr += 1
                    si["on_wait"] = [waits[-1]]
                new_insts.append(inst)
            bb["instructions"] = new_insts
    return _json.dumps(m).encode()


def _install_wait_splitter(nc: bass.Bass) -> None:
    orig = nc.to_json_bytes

    def patched():
        return _split_waits_json(orig())

    nc.to_json_bytes = patched


def build_kernel() -> bass.Bass:
    nc = bass.Bass()
    # hi quad-packed: row q*128+p holds [i=0..3][1024 b cols] bf16 (8KB/row)
    xhq = nc.dram_tensor("xhq", [NQ * 128, 4 * BC], FP16, kind="ExternalInput")
    # lo pair-packed: row t*128+p holds [i=0..1][1024 b cols] e4m3 (2KB/row)
    xlp = nc.dram_tensor("xlp", [NP * 128, 2 * BC], FP8, kind="ExternalInput")
    # sign(W1) bf16 quad-packed: row q*128+p holds [i=0..3][512 j] (4KB/row)
    w1h = nc.dram_tensor("w1h", [NQ * 128, 4 * H], FP16, kind="ExternalInput")
    # sign(W1)*2^-6 e4m3, all pair-tiles per partition: [t=0..15][i=0..1][512 j]
    w1l = nc.dram_tensor("w1l", [128, NP * 2 * H], FP8, kind="ExternalInput")
    b1p = nc.dram_tensor("b1p", [128, NJ], F32, kind="ExternalInput")
    # sign(W2) bf16: w2s[p, j*C+c] = sign(W2)[j*128+p, c]
    w2sd = nc.dram_tensor("w2sd", [128, NJ * C], BF16, kind="ExternalInput")
    # b2 replicated [128, 4, C] for the fused [128,4,10] bias add
    b2r4 = nc.dram_tensor("b2r4", [128, 4 * C], F32, kind="ExternalInput")
    # packed per-core output [p, bt*10+c]; host reorders to [1024, 10]
    out = nc.dram_tensor("out", [128, NBT * C], F32, kind="ExternalOutput")

    with _PatchedTileContext(nc) as tc:
        with (
            tc.tile_pool(name="consts", bufs=1) as consts,
            tc.tile_pool(name="w1hp", bufs=NQ) as w1h_pool,
            tc.tile_pool(name="xh", bufs=NQ) as xh_pool,
            tc.tile_pool(name="xl", bufs=NP) as xl_pool,
            tc.tile_pool(name="signh", bufs=NJ * NBC) as signh_pool,
            tc.tile_pool(name="psum", bufs=8, space="PSUM") as psum_pool,
            tc.tile_pool(name="smx", bufs=10) as smx_pool,
        ):
            # allocation order bc-major: pool ring slots 0-3 = bc0 banks,
            # 4-7 = bc1, so each bc's four psD re-allocations alias banks
            # already freed by that bc's own sign() reads.
            _ps = [
                psum_pool.tile([128, 512], F32, name="psB", tag="psB")
                for _ in range(NJ * NBC)
            ]
            psumB = [[_ps[bc * NJ + j] for bc in range(NBC)] for j in range(NJ)]

            # HAM warmup: PE sits idle ~4us while the first tiles land; a
            # dozen dummy matmuls (into bank 0, overwritten by the first real
            # start=True matmul) bring the clock gate to 2.4GHz beforehand.
            warm = consts.tile([128, 640], FP16, name="warm", tag="warm")
            nc.vector.memset(warm[:], 0.0)
            for _ in range(14):
                nc.tensor.matmul(
                    psumB[0][0][:], warm[:, :128], warm[:, 128:640],
                    start=True, stop=True,
                )

            def quad_in(q):
                w1t = w1h_pool.tile([128, 4 * 512], FP16, name="w1t", tag="w1t")
                xht = xh_pool.tile([128, 4 * BC], FP16, name="xht", tag="xht")
                roww = w1h[q * 128:(q + 1) * 128, :]
                rowx = xhq[q * 128:(q + 1) * 128, :]
                if q == 0:
                    # startup: i-granular, in consumption order, so MM i can
                    # begin while i+1 is still in flight
                    for i in range(4):
                        nc.sync.dma_start(
                            w1t[:, i * 512:(i + 1) * 512],
                            roww[:, i * 512:(i + 1) * 512],
                        )
                        nc.sync.dma_start(
                            xht[:, i * BC:(i + 1) * BC],
                            rowx[:, i * BC:(i + 1) * BC],
                        )
                elif q == 1:
                    nc.sync.dma_start(w1t[:], roww)
                    nc.sync.dma_start(xht[:, 0:2 * BC], rowx[:, 0:2 * BC])
                    nc.sync.dma_start(xht[:, 2 * BC:4 * BC], rowx[:, 2 * BC:4 * BC])
                else:
                    nc.sync.dma_start(w1t[:], roww)
                    nc.sync.dma_start(xht[:], rowx)
                return w1t, xht

            # ---- hi pass: bf16, f-major over all 8 banks ----
            b1_t = w2s = b2q = w1lt = None
            xlt = [None] * NP
            for q in range(NQ):
                if q == 0:
                    with tc.high_priority():
                        w1t, xht = quad_in(q)
                else:
                    w1t, xht = quad_in(q)
                if q == 1:
                    # constants: packed, one DMA each, after the startup
                    # quads' stream DMAs so they stay off the critical path
                    b1_t = consts.tile([128, NJ], F32, name="b1t", tag="b1t")
                    nc.sync.dma_start(b1_t[:], b1p[:, :])
                    w2s = consts.tile([128, NJ * C], BF16, name="w2s", tag="w2s")
                    nc.sync.dma_start(w2s[:], w2sd[:, :])
                    b2q = consts.tile([128, 4, C], F32, name="b2q", tag="b2q")
                    nc.sync.dma_start(b2q[:], b2r4[:, :])
                for i in range(4):
                    for j in range(NJ):
                        for bc in range(NBC):
                            nc.tensor.matmul(
                                psumB[j][bc][:],
                                w1t[:, i * 512 + j * 128:i * 512 + (j + 1) * 128],
                                xht[:, i * BC + bc * 512:i * BC + (bc + 1) * 512],
                                start=(q == 0 and i == 0),
                                stop=False,
                            )

            # lo-pass data: emitted after every hi-pass DMA — the hi pass is
            # DMA-bandwidth-tight through ~quad 3, and this data isn't
            # consumed until the hi pass drains (~60us in)
            w1lt = consts.tile([128, NP, 2, H], FP8, name="w1lt", tag="w1lt")
            half = NP * H
            nc.sync.dma_start(w1lt[:, 0:NP // 2], w1l[:, 0:half])
            nc.sync.dma_start(w1lt[:, NP // 2:NP], w1l[:, half:2 * half])
            for t in range(NP):
                xl = xl_pool.tile([128, 2, BC], FP8, name="xl", tag="xl")
                nc.sync.dma_start(xl[:], xlp[t * 128:(t + 1) * 128, :])
                xlt[t] = xl

            # ---- lo pass: fp8 DoubleRow, bank-major; sign/mm2/softmax
            #      overlap the remaining DR MMs ----
            collect = smx_pool.tile([128, NBT * C], F32, name="collect", tag="collect")
            for bc in range(NBC):
                signh = [None] * NJ
                for j in range(NJ):
                    for t in range(NP):
                        nc.tensor.matmul(
                            psumB[j][bc][:],
                            w1lt[:, t, :, j * 128:(j + 1) * 128],
                            xlt[t][:, :, bc * 512:(bc + 1) * 512],
                            start=False,
                            stop=(t == NP - 1),
                            perf_mode=mybir.MatmulPerfMode.DoubleRow,
                        )
                    s = signh_pool.tile([128, 512], BF16, name="signh", tag="signh")
                    nc.scalar.sign(s[:], psumB[j][bc][:], bias=b1_t[:, j:j + 1])
                    signh[j] = s
                for bt in range(4):
                    gbt = bc * 4 + bt
                    # one bank per b-tile (PSUM start=True zeroing is coarser
                    # than 40B, so logit groups can't share a bank); aliases a
                    # bank this bc's signs already freed
                    ps2 = psum_pool.tile([128, C], F32, name="psD", tag="psB")
                    for j in range(NJ):
                        nc.tensor.matmul(
                            ps2[:],
                            signh[j][:, bt * 128:(bt + 1) * 128],
                            w2s[:, j * C:(j + 1) * C],
                            start=(j == 0),
                            stop=(j == NJ - 1),
                        )
                    logits = smx_pool.tile([128, C], F32, name="logits", tag="logits")
                    nc.vector.tensor_add(logits[:], ps2[:], b2q[:, 0])
                    negmax = smx_pool.tile([128, 1], F32, name="negmax", tag="negmax")
                    nc.vector.reduce_max(
                        negmax[:], logits[:], axis=mybir.AxisListType.X, negate=True
                    )
                    e = smx_pool.tile([128, C], F32, name="e", tag="e")
                    ssum = smx_pool.tile([128, 1], F32, name="ssum", tag="ssum")
                    nc.scalar.activation(
                        e[:],
                        logits[:],
                        mybir.ActivationFunctionType.Exp,
                        bias=negmax[:],
                        accum_out=ssum[:],
                    )
                    rinv = smx_pool.tile([128, 1], F32, name="rinv", tag="rinv")
                    nc.vector.reciprocal(rinv[:], ssum[:])
                    nc.vector.tensor_scalar_mul(
                        collect[:, gbt * C:(gbt + 1) * C],
                        e[:],
                        rinv[:],
                    )
                    nc.sync.dma_start(
                        out[:, gbt * C:(gbt + 1) * C],
                        collect[:, gbt * C:(gbt + 1) * C],
                    )

    _install_wait_splitter(nc)
    return nc


_cached_nc = None


def _get_nc() -> bass.Bass:
    global _cached_nc
    if _cached_nc is None:
        _cached_nc = build_kernel()
    return _cached_nc


def kernel(inputs, W1, b1, W2, b2):
    x = np.ascontiguousarray(np.asarray(inputs, dtype=np.float32))
    W1 = np.asarray(W1, dtype=np.float32)
    b1 = np.asarray(b1, dtype=np.float32)
    W2 = np.asarray(W2, dtype=np.float32)
    b2 = np.asarray(b2, dtype=np.float32)

    w1s = np.where(W1 >= 0, np.float32(1.0), np.float32(-1.0))
    # [4096, 512] -> quad-packed [NQ*128, 4*512] bf16
    w1h_pack = np.ascontiguousarray(
        w1s.astype(np.float16)
        .reshape(NQ, 4, 128, H).transpose(0, 2, 1, 3).reshape(NQ * 128, 4 * H)
    )
    # lo-pass weights: sign(W1)*2^-6, per-partition [t][i][512 j]
    w1l_pack = np.ascontiguousarray(
        (w1s * W_LO).astype(ml_dtypes.float8_e4m3)
        .reshape(NP, 2, 128, H).transpose(2, 0, 1, 3).reshape(128, NP * 2 * H)
    )
    b1_pack = np.ascontiguousarray(b1.reshape(NJ, 128).T)
    w2s_pack = np.ascontiguousarray(
        np.where(W2 >= 0, np.float32(1.0), np.float32(-1.0))
        .astype(ml_dtypes.bfloat16)
        .reshape(NJ, 128, C).transpose(1, 0, 2).reshape(128, NJ * C)
    )
    b2_rep4 = np.ascontiguousarray(
        np.broadcast_to(b2.reshape(1, 1, C), (128, 4, C)).reshape(128, 4 * C)
    )

    in_maps = []
    for c in range(NCORES):
        xc_t = x[c * BC:(c + 1) * BC, :].T  # [F, BC] fp32
        hi = xc_t.astype(np.float16)
        lo = (xc_t - hi.astype(np.float32)) * LO_SCALE
        lo8 = lo.astype(ml_dtypes.float8_e4m3)
        xh_pack = np.ascontiguousarray(
            hi.reshape(NQ, 4, 128, BC).transpose(0, 2, 1, 3).reshape(NQ * 128, 4 * BC)
        )
        xl_pack = np.ascontiguousarray(
            lo8.reshape(NP, 2, 128, BC).transpose(0, 2, 1, 3).reshape(NP * 128, 2 * BC)
        )
        in_maps.append(
            {
                "xhq": xh_pack,
                "xlp": xl_pack,
                "w1h": w1h_pack,
                "w1l": w1l_pack,
                "b1p": b1_pack,
                "w2sd": w2s_pack,
                "b2r4": b2_rep4,
            }
        )

    nc = _get_nc()
    res = run_bass_kernel_spmd(nc, in_maps, core_ids=list(range(NCORES)))
    global last_results
    last_results = res
    parts = []
    for c in range(NCORES):
        oc = res.results[c]["out"]  # [128, NBT*C]
        parts.append(
            oc.reshape(128, NBT, C).transpose(1, 0, 2).reshape(BC, C)
        )
    return np.concatenate(parts, axis=0).astype(np.float32)


last_results = None


# revision 8
# speedup vs baseline: 1.2658x; 1.0032x over previous
"""Binarized-MLP (BNN) kernel for Trainium2, data-parallel over batch on 8 NeuronCores.

Reference computation:
    h      = x @ sign(W1) + b1          x:[8192,4096] W1:[4096,512]
    logits = sign(h) @ sign(W2) + b2    W2:[512,10]
    out    = softmax(logits)            [8192,10]

Per-core strategy (batch shard of 1024 rows):
  - The dominant matmul runs as a fp16 "hi" pass plus an fp8-e4m3 "lo"
    residual pass in DoubleRow perf mode (2 fp8 rows per PE cell, 256-row
    contraction per matmul). lo = (x - fp16(x)) * 256 is host-quantized to
    e4m3; the 1/64 un-scale is folded into the lo-pass stationary weights
    (sign(W1) * 2^-8, exact as e4m3 subnormal; verified exact on HW). Both passes accumulate into the same
    fp32 PSUM banks. End-to-end rel err ~9e-3 (gate 2e-2), measured in
    numpy with bit-identical host quantization.
  - Weights are sign()ed on the host (free) — no device-side weight prep.
  - hi pass: stationary = sign(W1) f-tile [128f x 128j], moving = xT hi
    f-tile [128f x 512b] -> 8 PSUM banks hold h [512j x 1024b]; 256 MMs,
    f-major, fully dense on TensorE.
  - lo pass: stationary = 3D AP [128, 2, 128] fp8 pair-tile, moving =
    [128, 2, 512] fp8 -> 128 DoubleRow MMs, bank-major so each PSUM bank
    finishes early and sign/mm2/softmax overlap the remaining MMs.
  - sign(h)+b1 fused into one ScalarE Sign-activation (bias=b1) from PSUM
    into bf16 SBUF tiles [j, b], the stationary operand of the 2nd matmul.
  - 2nd matmul accumulates incrementally after each sign (j-wise) into a
    single PSUM bank holding all 4 b-tiles' logits [128, 4, 10].
  - softmax: add b2 + segmented reduce_max over [128,4,10] once, then per
    b-tile: Exp(bias=-max, accum_out=sum) -> DVE reciprocal -> scalar-mul,
    with a per-b-tile output DMA so only ~5KB trails the last chain.
"""

import numpy as np
import ml_dtypes

import concourse.bass as bass
import concourse.tile as tile
from concourse import mybir
from concourse.bass_utils import run_bass_kernel_spmd
from bass_rust import ScopedClock, VectorClock

_CLEAR_SEMS = False

BF16 = mybir.dt.bfloat16
FP16 = mybir.dt.float16
F32 = mybir.dt.float32
FP8 = mybir.dt.float8e4

B, F, H, C = 8192, 4096, 512, 10
NCORES = 8
BC = B // NCORES          # 1024 batch rows per core
NF = F // 128             # 32 f-tiles (contraction)
NJ = H // 128             # 4 j-tiles (hidden)
NBC = BC // 512           # 2 moving-operand chunks of 512
NBT = BC // 128           # 8 output b-tiles
NQ = NF // 4              # 8 hi-pass quads (4 f-tiles per DMA)
NP = NF // 2              # 16 lo-pass pair-tiles (DoubleRow: 2 f-tiles/MM)

LO_SCALE = 256.0          # lo residual pre-scale (dodges e4m3 subnormals)
W_LO = 1.0 / LO_SCALE     # folded into lo-pass weights; 2^-8 exact in e4m3 (subnormal)


class _PatchedTileContext(tile.TileContext):
    """Workaround for the walrus build in this container only accepting one
    sem wait on a CTRL-type (Drain) instruction: spread the exit drain's
    per-proc waits across several drains with one wait each."""

    def _drain_and_barrier(self, tick_clock, wait_clock):
        gc = tick_clock.global_clock
        ticks = list(gc)
        nprocs = len(ticks)
        engines = [
            self.nc.sync,
            self.nc.gpsimd,
            self.nc.vector,
            self.nc.scalar,
            self.nc.tensor,
        ]
        # Cheap wait-carriers: one engine NOP per pending proc tick, spread
        # round-robin so the waits resolve in parallel (a DRAIN costs ~1us on
        # some engines; a NOP ~50ns).
        k = 0
        for i, t in enumerate(ticks):
            if t == 0:
                continue
            partial = [0] * nprocs
            partial[i] = t
            inst = engines[k % len(engines)].nop()
            k += 1
            wait_clock.add_sem_waits(
                inst.ins, ScopedClock({None: VectorClock(partial)})
            )
        self.nc.sync.drain()

        self.nc.all_engine_barrier()
        assert self.sems is not None
        popped = self.nc._tile_sem_poison_stack.pop()
        assert popped is self._sem_poison
        if _CLEAR_SEMS:
            # gpsimd-only cleanup (range-clear is a single op there); no
            # closing barrier — each engine halts after its own stream, and
            # NEFF completion waits for all engines anyway.
            self.nc.clear_and_free_semaphores(list(self.sems.allocated().values()))


def _split_waits_json(raw: bytes) -> bytes:
    """The walrus build in this container accepts at most ONE sem wait per
    instruction (bass's own wait_op asserts the same). Tile attaches several.
    Rewrite the serialized BIR: excess waits become standalone EventSemaphore
    wait instructions on the same engine immediately before the instruction —
    semantically identical, since the engine blocks there first."""
    import json as _json

    m = _json.loads(raw)
    ctr = 0
    for fn in m.get("functions", []):
        for bb in fn.get("blocks", []):
            insts = bb.get("instructions", [])
            new_insts = []
            for inst in insts:
                si = inst.get("sync_info")
                waits = si.get("on_wait") or [] if si else []
                if len(waits) > 1:
                    for w in waits[:-1]:
                        new_insts.append(
                            {
                                "debug": inst.get("debug", 0),
                                "engine": inst["engine"],
                                "ins": [],
                                "outs": [],
                                "name": f"WSPLIT-{ctr}",
                                "opcode": "EventSemaphore",
                                "sync_info": {"on_update": [], "on_wait": [w]},
                            }
                        )
                        ctr += 1
                    si["on_wait"] = [waits[-1]]
                new_insts.append(inst)
            bb["instructions"] = new_insts
    return _json.dumps(m).encode()


def _install_wait_splitter(nc: bass.Bass) -> None:
    orig = nc.to_json_bytes

    def patched():
        return _split_waits_json(orig())

    nc.to_json_bytes = patched


def build_kernel() -> bass.Bass:
    nc = bass.Bass()
    # hi quad-packed: row q*128+p holds [i=0..3][1024 b cols] bf16 (8KB/row)
    xhq = nc.dram_tensor("xhq", [NQ * 128, 4 * BC], FP16, kind="ExternalInput")
    # lo pair-packed: row t*128+p holds [i=0..1][1024 b cols] e4m3 (2KB/row)
    xlp = nc.dram_tensor("xlp", [NP * 128, 2 * BC], FP8, kind="ExternalInput")
    # sign(W1) bf16 quad-packed: row q*128+p holds [i=0..3][512 j] (4KB/row)
    w1h = nc.dram_tensor("w1h", [NQ * 128, 4 * H], FP16, kind="ExternalInput")
    # sign(W1)*2^-6 e4m3, all pair-tiles per partition: [t=0..15][i=0..1][512 j]
    w1l = nc.dram_tensor("w1l", [128, NP * 2 * H], FP8, kind="ExternalInput")
    b1p = nc.dram_tensor("b1p", [128, NJ], F32, kind="ExternalInput")
    # sign(W2) bf16: w2s[p, j*C+c] = sign(W2)[j*128+p, c]
    w2sd = nc.dram_tensor("w2sd", [128, NJ * C], BF16, kind="ExternalInput")
    # b2 replicated [128, 4, C] for the fused [128,4,10] bias add
    b2r4 = nc.dram_tensor("b2r4", [128, 4 * C], F32, kind="ExternalInput")
    # packed per-core output [p, bt*10+c]; host reorders to [1024, 10]
    out = nc.dram_tensor("out", [128, NBT * C], F32, kind="ExternalOutput")

    with _PatchedTileContext(nc) as tc:
        with (
            tc.tile_pool(name="consts", bufs=1) as consts,
            tc.tile_pool(name="w1hp", bufs=NQ) as w1h_pool,
            tc.tile_pool(name="xh", bufs=NQ) as xh_pool,
            tc.tile_pool(name="xl", bufs=NP) as xl_pool,
            tc.tile_pool(name="signh", bufs=NJ * NBC) as signh_pool,
            tc.tile_pool(name="psum", bufs=8, space="PSUM") as psum_pool,
            tc.tile_pool(name="smx", bufs=10) as smx_pool,
        ):
            # allocation order bc-major: pool ring slots 0-3 = bc0 banks,
            # 4-7 = bc1, so each bc's four psD re-allocations alias banks
            # already freed by that bc's own sign() reads.
            _ps = [
                psum_pool.tile([128, 512], F32, name="psB", tag="psB")
                for _ in range(NJ * NBC)
            ]
            psumB = [[_ps[bc * NJ + j] for bc in range(NBC)] for j in range(NJ)]

            # HAM warmup: PE sits idle ~4us while the first tiles land; a
            # dozen dummy matmuls (into bank 0, overwritten by the first real
            # start=True matmul) bring the clock gate to 2.4GHz beforehand.
            warm = consts.tile([128, 640], FP16, name="warm", tag="warm")
            nc.vector.memset(warm[:], 0.0)
            for _ in range(14):
                nc.tensor.matmul(
                    psumB[0][0][:], warm[:, :128], warm[:, 128:640],
                    start=True, stop=True,
                )

            def quad_in(q):
                w1t = w1h_pool.tile([128, 4 * 512], FP16, name="w1t", tag="w1t")
                xht = xh_pool.tile([128, 4 * BC], FP16, name="xht", tag="xht")
                roww = w1h[q * 128:(q + 1) * 128, :]
                rowx = xhq[q * 128:(q + 1) * 128, :]
                if q == 0:
                    # startup: i-granular, in consumption order, so MM i can
                    # begin while i+1 is still in flight
                    for i in range(4):
                        nc.sync.dma_start(
                            w1t[:, i * 512:(i + 1) * 512],
                            roww[:, i * 512:(i + 1) * 512],
                        )
                        nc.sync.dma_start(
                            xht[:, i * BC:(i + 1) * BC],
                            rowx[:, i * BC:(i + 1) * BC],
                        )
                elif q == 1:
                    nc.sync.dma_start(w1t[:], roww)
                    nc.sync.dma_start(xht[:, 0:2 * BC], rowx[:, 0:2 * BC])
                    nc.sync.dma_start(xht[:, 2 * BC:4 * BC], rowx[:, 2 * BC:4 * BC])
                else:
                    nc.sync.dma_start(w1t[:], roww)
                    nc.sync.dma_start(xht[:], rowx)
                return w1t, xht

            # ---- hi pass: bf16, f-major over all 8 banks ----
            b1_t = w2s = b2q = w1lt = None
            xlt = [None] * NP
            for q in range(NQ):
                if q == 0:
                    with tc.high_priority():
                        w1t, xht = quad_in(q)
                else:
                    w1t, xht = quad_in(q)
                if q == 1:
                    # constants: packed, one DMA each, after the startup
                    # quads' stream DMAs so they stay off the critical path
                    b1_t = consts.tile([128, NJ], F32, name="b1t", tag="b1t")
                    nc.sync.dma_start(b1_t[:], b1p[:, :])
                    w2s = consts.tile([128, NJ * C], BF16, name="w2s", tag="w2s")
                    nc.sync.dma_start(w2s[:], w2sd[:, :])
                    b2q = consts.tile([128, 4, C], F32, name="b2q", tag="b2q")
                    nc.sync.dma_start(b2q[:], b2r4[:, :])
                for i in range(4):
                    for j in range(NJ):
                        for bc in range(NBC):
                            nc.tensor.matmul(
                                psumB[j][bc][:],
                                w1t[:, i * 512 + j * 128:i * 512 + (j + 1) * 128],
                                xht[:, i * BC + bc * 512:i * BC + (bc + 1) * 512],
                                start=(q == 0 and i == 0),
                                stop=False,
                            )

            # lo-pass data: emitted after every hi-pass DMA — the hi pass is
            # DMA-bandwidth-tight through ~quad 3, and this data isn't
            # consumed until the hi pass drains (~60us in)
            w1lt = consts.tile([128, NP, 2, H], FP8, name="w1lt", tag="w1lt")
            half = NP * H
            nc.sync.dma_start(w1lt[:, 0:NP // 2], w1l[:, 0:half])
            nc.sync.dma_start(w1lt[:, NP // 2:NP], w1l[:, half:2 * half])
            for t in range(NP):
                xl = xl_pool.tile([128, 2, BC], FP8, name="xl", tag="xl")
                nc.sync.dma_start(xl[:], xlp[t * 128:(t + 1) * 128, :])
                xlt[t] = xl

            # ---- lo pass: fp8 DoubleRow, bank-major; sign/mm2/softmax
            #      overlap the remaining DR MMs ----
            collect = smx_pool.tile([128, NBT * C], F32, name="collect", tag="collect")
            for bc in range(NBC):
                signh = [None] * NJ
                for j in range(NJ):
                    for t in range(NP):
                        nc.tensor.matmul(
                            psumB[j][bc][:],
                            w1lt[:, t, :, j * 128:(j + 1) * 128],
                            xlt[t][:, :, bc * 512:(bc + 1) * 512],
                            start=False,
                            stop=(t == NP - 1),
                            perf_mode=mybir.MatmulPerfMode.DoubleRow,
                        )
                    s = signh_pool.tile([128, 512], BF16, name="signh", tag="signh")
                    nc.scalar.sign(s[:], psumB[j][bc][:], bias=b1_t[:, j:j + 1])
                    signh[j] = s
                for bt in range(4):
                    gbt = bc * 4 + bt
                    # one bank per b-tile (PSUM start=True zeroing is coarser
                    # than 40B, so logit groups can't share a bank); aliases a
                    # bank this bc's signs already freed
                    ps2 = psum_pool.tile([128, C], F32, name="psD", tag="psB")
                    for j in range(NJ):
                        nc.tensor.matmul(
                            ps2[:],
                            signh[j][:, bt * 128:(bt + 1) * 128],
                            w2s[:, j * C:(j + 1) * C],
                            start=(j == 0),
                            stop=(j == NJ - 1),
                        )
                    logits = smx_pool.tile([128, C], F32, name="logits", tag="logits")
                    nc.vector.tensor_add(logits[:], ps2[:], b2q[:, 0])
                    negmax = smx_pool.tile([128, 1], F32, name="negmax", tag="negmax")
                    nc.vector.reduce_max(
                        negmax[:], logits[:], axis=mybir.AxisListType.X, negate=True
                    )
                    e = smx_pool.tile([128, C], F32, name="e", tag="e")
                    ssum = smx_pool.tile([128, 1], F32, name="ssum", tag="ssum")
                    nc.scalar.activation(
                        e[:],
                        logits[:],
                        mybir.ActivationFunctionType.Exp,
                        bias=negmax[:],
                        accum_out=ssum[:],
                    )
                    rinv = smx_pool.tile([128, 1], F32, name="rinv", tag="rinv")
                    nc.vector.reciprocal(rinv[:], ssum[:])
                    nc.vector.tensor_scalar_mul(
                        collect[:, gbt * C:(gbt + 1) * C],
                        e[:],
                        rinv[:],
                    )
                    nc.sync.dma_start(
                        out[:, gbt * C:(gbt + 1) * C],
                        collect[:, gbt * C:(gbt + 1) * C],
                    )

    _install_wait_splitter(nc)
    return nc


_cached_nc = None


def _get_nc() -> bass.Bass:
    global _cached_nc
    if _cached_nc is None:
        _cached_nc = build_kernel()
    return _cached_nc


def kernel(inputs, W1, b1, W2, b2):
    x = np.ascontiguousarray(np.asarray(inputs, dtype=np.float32))
    W1 = np.asarray(W1, dtype=np.float32)
    b1 = np.asarray(b1, dtype=np.float32)
    W2 = np.asarray(W2, dtype=np.float32)
    b2 = np.asarray(b2, dtype=np.float32)

    w1s = np.where(W1 >= 0, np.float32(1.0), np.float32(-1.0))
    # [4096, 512] -> quad-packed [NQ*128, 4*512] bf16
    w1h_pack = np.ascontiguousarray(
        w1s.astype(np.float16)
        .reshape(NQ, 4, 128, H).transpose(0, 2, 1, 3).reshape(NQ * 128, 4 * H)
    )
    # lo-pass weights: sign(W1)*2^-6, per-partition [t][i][512 j]
    w1l_pack = np.ascontiguousarray(
        (w1s * W_LO).astype(ml_dtypes.float8_e4m3)
        .reshape(NP, 2, 128, H).transpose(2, 0, 1, 3).reshape(128, NP * 2 * H)
    )
    b1_pack = np.ascontiguousarray(b1.reshape(NJ, 128).T)
    w2s_pack = np.ascontiguousarray(
        np.where(W2 >= 0, np.float32(1.0), np.float32(-1.0))
        .astype(ml_dtypes.bfloat16)
        .reshape(NJ, 128, C).transpose(1, 0, 2).reshape(128, NJ * C)
    )
    b2_rep4 = np.ascontiguousarray(
        np.broadcast_to(b2.reshape(1, 1, C), (128, 4, C)).reshape(128, 4 * C)
    )

    in_maps = []
    for c in range(NCORES):
        xc_t = x[c * BC:(c + 1) * BC, :].T  # [F, BC] fp32
        hi = xc_t.astype(np.float16)
        lo = (xc_t - hi.astype(np.float32)) * LO_SCALE
        lo8 = lo.astype(ml_dtypes.float8_e4m3)
        xh_pack = np.ascontiguousarray(
            hi.reshape(NQ, 4, 128, BC).transpose(0, 2, 1, 3).reshape(NQ * 128, 4 * BC)
        )
        xl_pack = np.ascontiguousarray(
            lo8.reshape(NP, 2, 128, BC).transpose(0, 2, 1, 3).reshape(NP * 128, 2 * BC)
        )
        in_maps.append(
            {
                "xhq": xh_pack,
                "xlp": xl_pack,
                "w1h": w1h_pack,
                "w1l": w1l_pack,
                "b1p": b1_pack,
                "w2sd": w2s_pack,
                "b2r4": b2_rep4,
            }
        )

    nc = _get_nc()
    res = run_bass_kernel_spmd(nc, in_maps, core_ids=list(range(NCORES)))
    global last_results
    last_results = res
    parts = []
    for c in range(NCORES):
        oc = res.results[c]["out"]  # [128, NBT*C]
        parts.append(
            oc.reshape(128, NBT, C).transpose(1, 0, 2).reshape(BC, C)
        )
    return np.concatenate(parts, axis=0).astype(np.float32)


last_results = None
